# revision 1
# baseline (speedup 1.0000x reference)
"""CTRNN policy kernel for Trainium2 (8 NeuronCores, batch-parallel).

Reference computation (per batch element b, B=64, N=1024, OBS=64, A=16):
    I = E[b] @ obs[b]
    repeat ITERS x:  y = tanh(gain*(v+bias))*mask
                     v = (v + DT/tau * (-v + W[b]@y + I)) * mask
    action[b] = D[b] @ v

Sharding: batch 64 -> 8 cores x 8 individuals, fully data parallel.

Algebraic refactor (all folds on host):
    am = DT/tau*mask, cm = (1-DT/tau)*mask
    s  = g*(v+bias)                  (state; g = gain, zero-guarded)
    Wg = diag(g*am) W diag(mask)     -> bf16 on device (SBUF-resident)
    Ig = g*(am*(E@obs) + bias*(1-cm))
    per iteration: y = tanh(s);  s' = cm*s + Wg@y + Ig
    action = (D/g) @ s_final - D@bias

Per-core schedule: 2 rounds x 4 individuals. The matvec for the 4
individuals of a round runs on the 4 PE column strips (tile_position
col-tiling): stationary = y column [128,1] bf16, moving = Wg^T slab
[128,512] bf16, outputs land as rows [1,512] at PSUM partitions
{0,32,64,96} of 2 shared banks. VectorE reads each full PSUM bank
[128,512] (fusing the +Ig add; non-row partitions are dead lanes at no
extra cost), PE transposes [128,128] blocks back to column layout, and
the per-individual state update [128,8] reads the transposed columns
at free-stride 128. No DMA in the recurrent loop.

Column layout per individual: n = p + 128*t stored at tile[p, t].
"""

import os
import sys
from contextlib import ExitStack

import numpy as np

for _p in ("/opt/trn_rl_repo", "/root/.axon_site/_ro/trn_rl_repo"):
    if os.path.isdir(_p) and _p not in sys.path:
        sys.path.append(_p)

import concourse.bass as bass  # noqa: E402
import concourse.tile as tile  # noqa: E402
from concourse import bacc, mybir  # noqa: E402
from concourse.bass_utils import run_bass_kernel_spmd  # noqa: E402

DT = 0.1
ITERS = int(1.0 // DT)  # == 9: reference.py uses `int(1.0 // DT)`, and 1.0//0.1 == 9.0
B_FULL, N, OBS, ADIM = 64, 1024, 64, 16
NCORES = 8
BPC = B_FULL // NCORES  # individuals per core
P = 128
NCH = 8                 # 128-chunks per vector
RQ = 4                  # individuals per round (one per PE column strip)
NR = 2                  # rounds
F32 = mybir.dt.float32
F32R = mybir.dt.float32r
BF16 = mybir.dt.bfloat16


def make_pools(ctx, tc):
    return dict(
        const=ctx.enter_context(tc.tile_pool(name="const", bufs=1)),
        wpool=ctx.enter_context(tc.tile_pool(name="w", bufs=22)),
        state=ctx.enter_context(tc.tile_pool(name="state", bufs=2)),
        prow=ctx.enter_context(tc.tile_pool(name="prow", bufs=4, space="PSUM")),
        ptr=ctx.enter_context(tc.tile_pool(name="ptr", bufs=2, space="PSUM")),
    )


def kernel_body(ctx, tc, ins, out_ap, iters=ITERS, pools=None, w_sb=None):
    nc = tc.nc
    Tanh = mybir.ActivationFunctionType.Tanh
    add = mybir.AluOpType.add
    mult = mybir.AluOpType.mult
    sub = mybir.AluOpType.subtract

    p = pools if pools is not None else make_pools(ctx, tc)
    const, wpool, state = p["const"], p["wpool"], p["state"]
    prow, ptr = p["prow"], p["ptr"]

    # ---- constants ----
    ident_sb = const.tile([P, P], F32, tag="ident", name="ident")
    nc.gpsimd.dma_start(ident_sb[:], ins["ident"][:])
    cm_sb = const.tile([P, BPC * NCH], F32, tag="cm", name="cm")
    nc.gpsimd.dma_start(cm_sb[:], ins["cmc"][:])
    igp_sb = {}
    for r in range(NR):
        igp_sb[r] = const.tile([P, 2 * 512], F32, tag=f"ig{r}", name=f"ig{r}")
        nc.gpsimd.dma_start(igp_sb[r][:], ins["igp"][r])
    # (split variant keeps the single igp tile; only PSUM differs)

    # ---- initial state + y0, batched [128, 64]; per-individual slices ----
    s0_sb = const.tile([P, BPC * NCH], F32, tag="s0", name="s0")
    nc.gpsimd.dma_start(s0_sb[:], ins["s0c"][:])
    y0_sb = const.tile([P, BPC * NCH], BF16, tag="y0", name="y0")
    nc.scalar.activation(y0_sb[:], s0_sb[:], Tanh)
    s_cur = [s0_sb[:, NCH * b:NCH * b + NCH] for b in range(BPC)]
    y_cur = [y0_sb[:, NCH * b:NCH * b + NCH] for b in range(BPC)]

    # ---- W loads (resident for the whole loop; half-tiles in round-major
    # order so "first needed" matches "first freed": round-0 matmuls start
    # after 8MB, and the next call's first loads overlap this call's tail) ----
    if w_sb is None:
        w_sb = {}
        for half, b in [(hf, b) for rr in range(NR) for hf in range(2)
                        for b in range(RQ * rr, RQ * rr + RQ)]:
            if True:
                wt = wpool.tile([P, NCH * N // 2], BF16, tag="w", name=f"w{b}h{half}")
                nc.sync.dma_start(
                    wt[:], ins["Wsb"][b][:, half * (NCH * N // 2):
                                         (half + 1) * (NCH * N // 2)])
                w_sb[b, half] = wt

    # ---- decode constants (needed late; after W in DMA queue) ----
    dgt_sb = const.tile([P, BPC * P], F32, tag="dgt", name="dgt")
    nc.gpsimd.dma_start(dgt_sb[:], ins["dgtc"][:])
    db0_sb = const.tile([ADIM, BPC], F32, tag="db0", name="db0")
    nc.gpsimd.dma_start(db0_sb[:], ins["db0"][:])
    act_sb = const.tile([ADIM, BPC], F32, tag="act", name="act")

    # ---- recurrent loop ----
    for t in range(iters):
        # cm*s has no dependency on this iteration's matmuls; GpSimd is
        # otherwise idle in the loop (and SBUF-only, which this op is)
        tmp = [None] * BPC
        for b in range(BPC):
            tm = state.tile([P, NCH], F32, tag=f"t{b}", name=f"t{b}")
            nc.gpsimd.tensor_tensor(
                tm[:], cm_sb[:, NCH * b:NCH * b + NCH], s_cur[b][:], op=mult)
            tmp[b] = tm
        # matvec rows: 4 individuals per round on the 4 PE column strips
        prows = [None] * NR
        for r in range(NR):
            pr = [prow.tile([P, 512], F32, tag="pr", name="pr") for _ in range(2)]
            prows[r] = pr
            for h in range(NCH):
                for j in range(2):
                    for q in range(RQ):
                        b = RQ * r + q
                        off = (h % 4) * N + 512 * j
                        nc.tensor.matmul(
                            pr[j][32 * q:32 * q + 1, :],
                            y_cur[b][:, h:h + 1],
                            w_sb[b, h // 4][:, off:off + 512],
                            start=(h == 0), stop=(h == NCH - 1),
                            tile_position=(0, 32 * q),
                        )
        # full-bank PSUM read fused with +Ig -> SBUF [128, 512] (rows at 32q)
        us = [None] * NR
        for r in range(NR):
            u = [None, None]
            for j in range(2):
                u[j] = state.tile([P, 512], F32, tag=f"u{r}{j}", name=f"u{r}{j}")
                nc.vector.tensor_tensor(
                    u[j][:], prows[r][j][:],
                    igp_sb[r][:, 512 * j:512 * j + 512], op=add)
            us[r] = u
        # PE transposes: [128,128] blocks; chunk t of individual q lands at
        # column 32q + 128t of pt, so one stride-128 AP per individual below
        pts = [None] * NR
        for r in range(NR):
            pt = ptr.tile([P, 2 * 512], F32, tag="pt", name="pt")
            for t8 in range(NCH):
                nc.tensor.transpose(
                    pt[:, 128 * t8: 128 * t8 + 128],
                    us[r][t8 // 4][:, 128 * (t8 % 4):128 * (t8 % 4) + 128],
                    ident_sb[:],
                )
            pts[r] = pt
        # per-individual state update + tanh
        for r in range(NR):
            for q in range(RQ):
                b = RQ * r + q
                s_n = state.tile([P, NCH], F32, tag=f"s{b}", name=f"s{b}")
                nc.vector.tensor_tensor(
                    s_n[:],
                    tmp[b][:],
                    pts[r][:, 32 * q: 32 * q + 897:128],
                    op=add,
                )
                s_cur[b] = s_n
                if t < iters - 1:
                    y_n = state.tile([P, NCH], BF16, tag=f"y{b}", name=f"y{b}")
                    nc.scalar.activation(y_n[:], s_n[:], Tanh)
                    y_cur[b] = y_n

    # ---- decode: action = Dg @ s_final - D@bias ----
    # borrow a transpose-pool PSUM tile; individual b accumulates in column b
    pdt = ptr.tile([P, 2 * 512], F32, tag="pt", name="pt")
    pd = pdt[0:ADIM, 0:BPC]
    for b in range(BPC):
        for h in range(NCH):
            nc.tensor.matmul(
                pd[:, b:b + 1],
                dgt_sb[:, P * b + ADIM * h: P * b + ADIM * h + ADIM],
                s_cur[b][:, h:h + 1],
                start=(h == 0), stop=(h == NCH - 1),
            )
    nc.vector.tensor_tensor(act_sb[:], pd[:], db0_sb[:], op=sub)
    nc.sync.dma_start(out_ap[:], act_sb[:])


def build_nc(iters=ITERS, reps=1, w_once=False):
    nc = bacc.Bacc(
        "TRN2", target_bir_lowering=False, debug=False, enable_asserts=False,
    )
    ins = {}
    for name, shape, dt in [
        ("Wsb", [BPC, P, NCH * N], BF16),
        ("s0c", [P, BPC * NCH], F32),
        ("cmc", [P, BPC * NCH], F32),
        ("igp", [NR, P, 2 * 512], F32),
        ("dgtc", [P, BPC * P], F32),
        ("db0", [ADIM, BPC], F32),
        ("ident", [P, P], F32),
    ]:
        ins[name] = nc.dram_tensor(name, shape, dt, kind="ExternalInput").ap()
    out_ap = nc.dram_tensor("act", [ADIM, BPC], F32, kind="ExternalOutput").ap()

    with tile.TileContext(nc) as tc:
        with ExitStack() as ctx:
            pools = make_pools(ctx, tc)
            w_shared = None
            if w_once:
                nc0 = tc.nc
                w_shared = {}
                for half in range(2):
                    for b in range(BPC):
                        wt = pools["wpool"].tile(
                            [P, NCH * N // 2], BF16, tag="w", name=f"w{b}h{half}")
                        nc0.sync.dma_start(
                            wt[:], ins["Wsb"][b][:, half * (NCH * N // 2):
                                                 (half + 1) * (NCH * N // 2)])
                        w_shared[b, half] = wt
            for _rep in range(reps):
                kernel_body(ctx, tc, ins, out_ap, iters, pools, w_sb=w_shared)
    nc.compile()
    return nc


def prep_in_maps(obs, v0, tau, gain, bias, W, mask, E, D):
    f = np.float32
    obs, v0, tau, gain, bias, W, mask, E, D = [
        np.asarray(x, dtype=f) for x in (obs, v0, tau, gain, bias, W, mask, E, D)
    ]
    import ml_dtypes
    bf16 = ml_dtypes.bfloat16

    g = np.where(gain == 0.0, f(1e-6), gain)    # exact-rescaling guard
    am = (DT / tau) * mask                      # [64, N]
    cm = (1.0 - DT / tau) * mask
    I = np.einsum("bno,bo->bn", E, obs)         # [64, N]
    Ig = g * (am * I + bias * (1.0 - cm))
    s0 = g * (v0 + bias)
    Wg = W * (g * am)[:, :, None] * mask[:, None, :]
    # device layout: w[b][k, h*N + n] = Wg[b, n, 128h+k]
    WgT = Wg.transpose(0, 2, 1)                 # [b, m, n]
    wdev = np.ascontiguousarray(
        WgT.reshape(B_FULL, NCH, P, N).transpose(0, 2, 1, 3)
    ).reshape(B_FULL, P, NCH * N).astype(bf16)

    def cols(x):  # [64, N] -> [core, p, 8*b_local + t]  (n = p + 128 t)
        xc = x.reshape(NCORES, BPC, NCH, P)
        return np.ascontiguousarray(xc.transpose(0, 3, 1, 2)).reshape(
            NCORES, P, BPC * NCH)

    s0c = cols(s0)
    cmc = cols(cm)
    # padded Ig rows: igp[core, r, j, 32q, :] = Ig[8core+4r+q, 512j:512j+512]
    igp = np.zeros((NCORES, NR, P, 2 * 512), f)
    for r in range(NR):
        for j in range(2):
            for q in range(RQ):
                igp[:, r, 32 * q, 512 * j:512 * j + 512] = Ig.reshape(
                    NCORES, BPC, N)[:, RQ * r + q, 512 * j:512 * j + 512]
    Dg = D / g[:, None, :]
    dgt = np.ascontiguousarray(
        Dg.transpose(0, 2, 1).reshape(B_FULL, NCH, P, ADIM).transpose(0, 2, 1, 3)
    ).reshape(B_FULL, P, P)
    dgtc = np.ascontiguousarray(
        dgt.reshape(NCORES, BPC, P, P).transpose(0, 2, 1, 3)
    ).reshape(NCORES, P, BPC * P)
    db0 = np.einsum("ban,bn->ba", D, bias)
    db0c = np.ascontiguousarray(db0.reshape(NCORES, BPC, ADIM).transpose(0, 2, 1))
    ident = np.eye(P, dtype=f)

    in_maps = []
    for core in range(NCORES):
        s = slice(core * BPC, (core + 1) * BPC)
        in_maps.append({
            "Wsb": np.ascontiguousarray(wdev[s]),
            "s0c": s0c[core], "cmc": cmc[core], "igp": igp[core],
            "dgtc": dgtc[core], "db0": db0c[core], "ident": ident,
        })
    return in_maps


_NC_CACHE = None


def _get_nc():
    global _NC_CACHE
    if _NC_CACHE is None:
        _NC_CACHE = build_nc()
    return _NC_CACHE


def kernel(obs, v0, tau, gain, bias, W, mask, E, D):
    nc = _get_nc()
    in_maps = prep_in_maps(obs, v0, tau, gain, bias, W, mask, E, D)
    res = run_bass_kernel_spmd(nc, in_maps, core_ids=list(range(NCORES)))
    # device output is [ADIM, BPC] per core
    return np.concatenate(
        [np.ascontiguousarray(res.results[c]["act"].T) for c in range(NCORES)],
        axis=0,
    )

